# revision 6
# baseline (speedup 1.0000x reference)
"""Trainium2 Bass kernel for nn_NSF_prior (neural spline flow prior forward).

Strategy:
- 8-way data parallel over N=2^21 points (262144 per core), SPMD.
- Per core: loop over megatiles of NT = 128*NF points. Layout A = points on
  partitions ([128, NF, k] tiles, slot-major: point = base + slot*128 + p).
  Layout B = features on partitions for the tiny MLPs, 4-slot packed
  ([128, cols] tiles via block-diagonal weights).
- A->B via batched PE transposes; B->A via matmul with activations as lhsT
  (output lands points-on-partitions directly).
- Rational-quadratic spline: searchsorted as mask compares against
  normalized cumsum edges (one segmented scan per pass), gathers as
  one-hot-indicator dot products (mult + reduce), softplus only on the two
  gathered derivative params.
- sin/cos via fp32 magic-constant range reduction + ACT Sin (sets:
  trig_and_small <-> natural_log_exp_and_others only).
"""
import sys
import math
import os

sys.path.insert(0, "/opt/trn_rl_repo")

import numpy as np
import concourse.bass as bass
import concourse.tile as tile
from concourse import bacc, mybir
from concourse.bass_utils import run_bass_kernel_spmd
from concourse.masks import make_identity
from contextlib import ExitStack

F32 = mybir.dt.float32
I32 = mybir.dt.int32
ALU = mybir.AluOpType
AF = mybir.ActivationFunctionType
AX = mybir.AxisListType

NCORES = 8
NTOT = 2097152
P = 128
K = 20
MIN_BW = 0.001
MIN_D = 0.001
TB = 1.0
TAIL_CONST = float(np.float32(math.log(math.exp(1.0 - MIN_D) - 1.0)))
MAGIC = float(np.float32(1.5 * 2.0**23))
INV2PI = 1.0 / (2.0 * math.pi)
TWOPI = float(np.float32(2.0 * math.pi))
HALFPI = float(np.float32(math.pi / 2.0))
LOG2PI = float(np.float32(math.log(2.0 * math.pi)))
SCALE_W = float(np.float32(1.0 - MIN_BW * K))  # 0.98

FREQS4 = [1.0, 10.0 / 3.0, 17.0 / 3.0, 8.0]  # linspace(1, 8, 4)
FREQS5 = [1.0, 2.0, 4.0, 8.0, 16.0]  # 2^linspace(0, 4, 5)

# my xin column order -> reference row order (see proto.py validation)
P1 = [0, 1, 3, 5, 7, 2, 4, 6, 8]


def _perm_cond_e():
    p = [0, 1]
    for d in range(2):
        for f in range(5):
            p.append(2 + 4 * f + d)
    for d in range(2):
        for f in range(5):
            p.append(4 + 4 * f + d)
    p += list(range(22, 30))
    return p


PC = _perm_cond_e()
ROWPERM = P1 + [9 + c for c in PC]  # 39 entries


def f32(x):
    return np.ascontiguousarray(x, np.float32)


def block_diag4(b):
    """[r, c] block -> [4r, 4c] block-diagonal, 4 copies."""
    r, c = b.shape
    out = np.zeros((4 * r, 4 * c), np.float32)
    for i in range(4):
        out[i * r:(i + 1) * r, i * c:(i + 1) * c] = b
    return out


def host_consts(inp):
    """Pack weights/constants on the host. Returns dict name -> np array."""
    C = {}
    for tag, pre in (("1", "m1"), ("2", "m0")):
        w0 = f32(inp[f"{pre}_w0"])[ROWPERM, :]       # [39, 32]
        w1 = f32(inp[f"{pre}_w1"])                    # [32, 32]
        w2 = f32(inp[f"{pre}_w2"])                    # [32, 59]
        b0 = f32(inp[f"{pre}_b0"])
        b1 = f32(inp[f"{pre}_b1"])
        b2 = f32(inp[f"{pre}_b2"])
        C[f"w0m_{tag}"] = block_diag4(w0[0:32, :])    # [128, 128]
        C[f"w0r_{tag}"] = block_diag4(w0[32:39, :])   # [28, 128]
        C[f"w1d_{tag}"] = block_diag4(w1)             # [128, 128]
        C[f"w2b_{tag}"] = block_diag4(w2)             # [128, 236]
        C[f"b0c_{tag}"] = np.tile(b0, 4)[:, None]     # [128, 1]
        C[f"b1c_{tag}"] = np.tile(b1, 4)[:, None]
        C[f"eb2_{tag}"] = np.tile(np.exp(b2[:40])[None, :], (P, 1))  # [128,40]
        C[f"b2u_{tag}"] = np.tile(b2[40:59][None, :], (P, 1))        # [128,19]
    p0w0 = f32(inp["p0_w0"])[PC, :]                   # [30, 32]
    blk = np.zeros((32, 32), np.float32)
    blk[9:32, :] = p0w0[0:23, :]                      # xin cols 9..31
    C["p0m"] = block_diag4(blk)
    C["p0r"] = block_diag4(p0w0[23:30, :])            # xin cols 32..38
    C["p0w1"] = block_diag4(f32(inp["p0_w1"]))        # [128, 16]
    C["b0cp"] = np.tile(f32(inp["p0_b0"]), 4)[:, None]
    C["b1pc"] = np.tile(f32(inp["p0_b1"])[None, :], (P, 1))  # [128, 4]
    C["kbc"] = np.tile((np.arange(1, 20, dtype=np.float32) * MIN_BW)[None, :],
                       (P, 1))                        # [128, 19]
    return C


def reset_mask(nf):
    """Scan reset mask [128, nf*40]: 0 at segment starts (j*40, j*40+20)."""
    m = np.ones((P, nf * 40), np.float32)
    m[:, 0::40] = 0.0
    m[:, 20::40] = 0.0
    return m


CONST_SHAPES = {
    "w0m_1": (128, 128), "w0r_1": (28, 128), "w1d_1": (128, 128),
    "w2b_1": (128, 236), "b0c_1": (128, 1), "b1c_1": (128, 1),
    "eb2_1": (128, 40), "b2u_1": (128, 19),
    "w0m_2": (128, 128), "w0r_2": (28, 128), "w1d_2": (128, 128),
    "w2b_2": (128, 236), "b0c_2": (128, 1), "b1c_2": (128, 1),
    "eb2_2": (128, 40), "b2u_2": (128, 19),
    "p0m": (128, 128), "p0r": (28, 128), "p0w1": (128, 16),
    "b0cp": (128, 1), "b1pc": (128, 4), "kbc": (128, 19),
}


def register_const(nc, val):
    t = nc.alloc_sbuf_tensor(f"constf32-{val}", [128, 1], F32)
    nc.gpsimd.memset(t.ap(), val)
    nc.const_aps.aps[(F32, val)] = t.ap()
    nc.all_engine_barrier()


def build_nc(npc, nf):
    """Build the SPMD per-core program. npc points per core, nf slots/megatile."""
    NT = P * nf                 # points per megatile
    NMT = npc // NT
    assert npc % NT == 0 and nf % 4 == 0
    NG = nf // 4                # transpose/L2 groups per megatile
    M = nf * 32                 # B-layout columns per megatile
    NS = M // 512               # MLP strips per megatile
    assert M % 512 == 0

    nc = bacc.Bacc()
    register_const(nc, HALFPI)

    wi = nc.declare_dram_parameter("wi", [npc, 2], F32, isOutput=False)
    cond = nc.declare_dram_parameter("cond", [npc, 10], F32, isOutput=False)
    cst = {}
    for name, shp in CONST_SHAPES.items():
        cst[name] = nc.declare_dram_parameter(name, list(shp), F32, isOutput=False)
    rstc = nc.declare_dram_parameter("rstc", [P, nf * 40], F32, isOutput=False)
    wi_out = nc.declare_dram_parameter("wi_out", [npc, 2], F32, isOutput=True)
    log_pdf = nc.declare_dram_parameter("log_pdf", [npc, 1], F32, isOutput=True)

    V, A, G = nc.vector, nc.scalar, nc.gpsimd

    with ExitStack() as ctx:
        tc = ctx.enter_context(tile.TileContext(nc))
        cpool = ctx.enter_context(tc.tile_pool(name="consts", bufs=1))
        apool = ctx.enter_context(tc.tile_pool(name="at", bufs=2))
        wpool = ctx.enter_context(tc.tile_pool(name="work", bufs=1))
        spool = ctx.enter_context(tc.tile_pool(name="scal", bufs=1))
        bpool = ctx.enter_context(tc.tile_pool(name="bt", bufs=1))
        pst = ctx.enter_context(tc.tile_pool(name="pst", bufs=2, space="PSUM"))
        psr = ctx.enter_context(tc.tile_pool(name="psr", bufs=1, space="PSUM"))
        psm = ctx.enter_context(tc.tile_pool(name="psm", bufs=2, space="PSUM"))
        ptp = ctx.enter_context(tc.tile_pool(name="ptp", bufs=2, space="PSUM"))

        # --- constants in SBUF
        ct = {}
        for name, shp in CONST_SHAPES.items():
            ct[name] = cpool.tile(list(shp), F32, tag=f"c_{name}",
                                  name=f"ct_{name}")
            nc.sync.dma_start(ct[name][:], cst[name][:])
        rst_t = cpool.tile([P, nf * 40], F32, tag="c_rst")
        nc.sync.dma_start(rst_t[:], rstc[:])
        ident = cpool.tile([P, P], F32, tag="c_id")
        make_identity(nc, ident[:])
        fc4 = cpool.tile([P, 4], F32, tag="c_fc4")
        for i, f in enumerate(FREQS4):
            G.memset(fc4[:, i:i + 1], float(np.float32(f * INV2PI)))
        fc5 = cpool.tile([P, 5], F32, tag="c_fc5")
        for i, f in enumerate(FREQS5):
            G.memset(fc5[:, i:i + 1], float(np.float32(f * INV2PI)))

        def bc(ap, shape):
            return ap.broadcast_to(shape)

        def sincos_chains(xcol, xm, base):
            """x [128, nf] -> sin/cos features into xm[:, :, base:base+8].

            cols base..base+3 = sin(f*x), base+4..base+7 = cos(f*x)."""
            t = wpool.tile([P, nf, 4], F32, tag="ch", bufs=3, name="ch_t")
            m = wpool.tile([P, nf, 4], F32, tag="ch", bufs=3, name="ch_m")
            k = wpool.tile([P, nf, 4], F32, tag="ch", bufs=3, name="ch_k")
            r = wpool.tile([P, nf, 4], F32, tag="ch", bufs=3, name="ch_r")
            ra = wpool.tile([P, nf, 4], F32, tag="ch", bufs=3, name="ch_ra")
            xb = xcol.unsqueeze(2).broadcast_to((P, nf, 4))
            fb = fc4[:].unsqueeze(1).broadcast_to((P, nf, 4))
            V.tensor_tensor(t[:], xb, fb, ALU.mult)
            A.activation(m[:], t[:], AF.Copy, bias=MAGIC)
            A.activation(k[:], m[:], AF.Copy, bias=-MAGIC)
            V.tensor_tensor(r[:], t[:], k[:], ALU.subtract)
            V.tensor_scalar(ra[:].bitcast(I32), r[:].bitcast(I32),
                            0x7FFFFFFF, None, ALU.bitwise_and)
            A.activation(xm[:, :, base:base + 4], r[:], AF.Sin, scale=TWOPI)
            A.activation(xm[:, :, base + 4:base + 8], ra[:], AF.Sin,
                         scale=-TWOPI, bias=HALFPI)

        def cond_chains(xm):
            """c8, c9 (= xm cols 9:11) -> sin/cos into xm cols 11..31."""
            cpe = xm[:, :, 9:11]  # [P, nf, 2]
            t = wpool.tile([P, nf, 2, 5], F32, tag="cc", bufs=3, name="cc_t")
            m = wpool.tile([P, nf, 2, 5], F32, tag="cc", bufs=3, name="cc_m")
            k = wpool.tile([P, nf, 2, 5], F32, tag="cc", bufs=3, name="cc_k")
            r = wpool.tile([P, nf, 2, 5], F32, tag="cc", bufs=3, name="cc_r")
            ra = wpool.tile([P, nf, 2, 5], F32, tag="cc", bufs=3, name="cc_ra")
            xb = cpe.unsqueeze(3).broadcast_to((P, nf, 2, 5))
            fb = fc5[:].unsqueeze(1).unsqueeze(1).broadcast_to((P, nf, 2, 5))
            V.tensor_tensor(t[:], xb, fb, ALU.mult)
            A.activation(m[:], t[:], AF.Copy, bias=MAGIC)
            A.activation(k[:], m[:], AF.Copy, bias=-MAGIC)
            V.tensor_tensor(r[:], t[:], k[:], ALU.subtract)
            V.tensor_scalar(ra[:].bitcast(I32), r[:].bitcast(I32),
                            0x7FFFFFFF, None, ALU.bitwise_and)
            A.activation(xm[:, :, 11:21].rearrange("p a (b c) -> p a b c", b=2),
                         r[:], AF.Sin, scale=TWOPI)
            A.activation(xm[:, :, 21:31].rearrange("p a (b c) -> p a b c", b=2),
                         ra[:], AF.Sin, scale=-TWOPI, bias=HALFPI)

        def transpose_to_b(xm, xr, xmB, xrB):
            """A->B: [128, nf, 32]+[128, nf, 7] -> [128, M]+[28, M] packed."""
            for s in range(NS):
                pm = pst.tile([P, 512], F32, tag="trm")
                pr = psr.tile([28, 512], F32, tag="trr")
                for gg in range(4):
                    g = 4 * s + gg
                    src_m = xm[:, 4 * g:4 * g + 4, :].rearrange("p a b -> p (a b)")
                    nc.tensor.transpose(pm[:, 128 * gg:128 * (gg + 1)], src_m,
                                        ident[:])
                    src_r = xr[:, 4 * g:4 * g + 4, :].rearrange("p a b -> p (a b)")
                    nc.tensor.transpose(pr[:, 128 * gg:128 * (gg + 1)], src_r,
                                        ident[:, 0:28]
                                        if False else ident[:])
                A.copy(xmB[:, 512 * s:512 * (s + 1)], pm[:])
                V.tensor_copy(xrB[:, 512 * s:512 * (s + 1)], pr[:])

        def mlp(xmB, xrB, w0m, w0r, w1d, b0c, b1c, h1B):
            for s in range(NS):
                sl = slice(512 * s, 512 * (s + 1))
                h0p = psm.tile([P, 512], F32, tag="mm")
                nc.tensor.matmul(h0p[:], w0m[:], xmB[:, sl], start=True,
                                 stop=False)
                nc.tensor.matmul(h0p[:], w0r[:], xrB[:, sl], start=False,
                                 stop=True)
                h0s = bpool.tile([P, 512], F32, tag="h0", bufs=2)
                A.activation(h0s[:], h0p[:], AF.Prelu, bias=b0c[:, 0:1],
                             alpha=0.01)
                h1p = psm.tile([P, 512], F32, tag="mm")
                nc.tensor.matmul(h1p[:], w1d[:], h0s[:], start=True, stop=True)
                A.activation(h1B[:, sl], h1p[:], AF.Prelu, bias=b1c[:, 0:1],
                             alpha=0.01)

        def rqs_pass(tag, h1B, w2b, eb2, b2u, x_ap, y_out_ap, ld_tile, xm):
            """Spline pass. x_ap: [128, nf] raw input; writes y into y_out_ap
            ([128, nf] strided ap) and log-abs-det into ld_tile [128, nf]."""
            E = wpool.tile([P, nf, 40], F32, tag="E")
            cums = wpool.tile([P, nf, 40], F32, tag="cums")
            ud21 = wpool.tile([P, nf, 21], F32, tag="ud21")
            mful = wpool.tile([P, nf, 21], F32, tag="mful")
            ind = wpool.tile([P, nf, 20], F32, tag="ind")
            uv = wpool.tile([P, nf, 2, 19], F32, tag="uv")
            tpps = []
            for g in range(NG):
                tpp = ptp.tile([P, 236], F32, tag="tp")
                nc.tensor.matmul(tpp[:], h1B[:, 128 * g:128 * (g + 1)],
                                 w2b[:], start=True, stop=True)
                tpv = tpp[:].rearrange("p (a b) -> p a b", b=59)
                sl = slice(4 * g, 4 * (g + 1))
                A.activation(E[:, sl, :], tpv[:, :, 0:40], AF.Exp)
                V.scalar_tensor_tensor(
                    ud21[:, sl, 1:20], tpv[:, :, 40:59], 1.0,
                    bc(b2u[:].unsqueeze(1), (P, 4, 19)), ALU.mult, ALU.add)
                tpps.append(tpp)
            # E *= exp(b2[:40]) (fold layer-2 bias on uw|uh)
            G.tensor_tensor(E[:], E[:], bc(eb2[:].unsqueeze(1), (P, nf, 40)),
                            ALU.mult)
            G.memset(ud21[:, :, 0:1], TAIL_CONST)
            G.memset(ud21[:, :, 20:21], TAIL_CONST)
            # segmented cumsum over (uw|uh) exp values
            V.tensor_tensor_scan(
                cums[:].rearrange("p a b -> p (a b)"), rst_t[:],
                E[:].rearrange("p a b -> p (a b)"), 0.0, ALU.mult, ALU.add)
            rsw = spool.tile([P, nf], F32, tag=f"rsw")
            rsh = spool.tile([P, nf], F32, tag=f"rsh")
            V.reciprocal(rsw[:], cums[:, :, 19])
            V.reciprocal(rsh[:], cums[:, :, 39])
            xc = spool.tile([P, nf], F32, tag="xc")
            V.tensor_scalar(xc[:], x_ap, -TB, TB, ALU.max, ALU.min)
            ap = spool.tile([P, nf], F32, tag="ap_")
            A.activation(ap[:], xc[:], AF.Copy, scale=0.5, bias=0.5)
            # u/v = normalized interior edges (k = 1..19)
            kb = bc(ct["kbc"][:].unsqueeze(1), (P, nf, 19))
            t1 = wpool.tile([P, nf, 19], F32, tag="t1")
            V.tensor_tensor(t1[:], cums[:, :, 0:19],
                            bc(rsw[:].unsqueeze(2), (P, nf, 19)), ALU.mult)
            V.scalar_tensor_tensor(uv[:, :, 0, :], t1[:], SCALE_W, kb,
                                   ALU.mult, ALU.add)
            t1h = wpool.tile([P, nf, 19], F32, tag="t1")
            V.tensor_tensor(t1h[:], cums[:, :, 20:39],
                            bc(rsh[:].unsqueeze(2), (P, nf, 19)), ALU.mult)
            V.scalar_tensor_tensor(uv[:, :, 1, :], t1h[:], SCALE_W, kb,
                                   ALU.mult, ALU.add)
            # mask + indicator
            G.memset(mful[:, :, 0:1], 1.0)
            G.memset(mful[:, :, 20:21], 0.0)
            V.tensor_tensor(mful[:, :, 1:20], uv[:, :, 0, :],
                            bc(ap[:].unsqueeze(2), (P, nf, 19)), ALU.is_le)
            G.tensor_tensor(ind[:], mful[:, :, 0:20], mful[:, :, 1:21],
                            ALU.subtract)
            # dots
            indb2 = ind[:].unsqueeze(2).broadcast_to((P, nf, 2, 20))
            mulE = wpool.tile([P, nf, 2, 20], F32, tag="muls", bufs=2)
            V.tensor_tensor(mulE[:], E[:].rearrange("p a (b c) -> p a b c", b=2),
                            indb2, ALU.mult)
            gE = spool.tile([P, nf, 2], F32, tag="gE")
            V.tensor_reduce(gE[:], mulE[:], AX.X, ALU.add)
            mud = wpool.tile([P, nf, 2, 20], F32, tag="muls", bufs=2)
            G.tensor_tensor(mud[:, :, 0, :], ud21[:, :, 0:20], ind[:], ALU.mult)
            G.tensor_tensor(mud[:, :, 1, :], ud21[:, :, 1:21], ind[:], ALU.mult)
            gud = spool.tile([P, nf, 2], F32, tag="gud")
            V.tensor_reduce(gud[:], mud[:], AX.X, ALU.add)
            muv = wpool.tile([P, nf, 2, 19], F32, tag="muls", bufs=2)
            ind1b = ind[:, :, 1:20].unsqueeze(2).broadcast_to((P, nf, 2, 19))
            G.tensor_tensor(muv[:], uv[:], ind1b, ALU.mult)
            guv = spool.tile([P, nf, 2], F32, tag="guv")
            V.tensor_reduce(guv[:], muv[:], AX.X, ALU.add)
            # gathered scalars
            incw = spool.tile([P, nf], F32, tag="incw")
            A.activation(incw[:], guv[:, :, 0], AF.Copy, scale=2.0, bias=-1.0)
            inch = spool.tile([P, nf], F32, tag="inch")
            A.activation(inch[:], guv[:, :, 1], AF.Copy, scale=2.0, bias=-1.0)
            inwh = spool.tile([P, nf, 2], F32, tag="inwh")
            t2 = spool.tile([P, nf, 2], F32, tag="t2_")
            rswh = spool.tile([P, nf, 2], F32, tag="rswh")
            V.tensor_copy(rswh[:, :, 0], rsw[:])
            V.tensor_copy(rswh[:, :, 1], rsh[:])
            V.tensor_tensor(t2[:], gE[:], rswh[:], ALU.mult)
            A.activation(inwh[:], t2[:], AF.Copy, scale=2.0 * SCALE_W,
                         bias=2.0 * MIN_BW)
            # softplus on gathered ud
            esp = spool.tile([P, nf, 2], F32, tag="esp")
            A.activation(esp[:], gud[:], AF.Exp)
            d01 = spool.tile([P, nf, 2], F32, tag="d01")
            A.activation(d01[:], esp[:], AF.Ln, bias=1.0)
            V.tensor_scalar(d01[:], d01[:], MIN_D, None, ALU.add)
            # spline formula
            rinw = spool.tile([P, nf], F32, tag="rinw")
            V.reciprocal(rinw[:], inwh[:, :, 0])
            indel = spool.tile([P, nf], F32, tag="indel")
            V.tensor_tensor(indel[:], inwh[:, :, 1], rinw[:], ALU.mult)
            tmp = spool.tile([P, nf], F32, tag="tmp")
            G.tensor_tensor(tmp[:], xc[:], incw[:], ALU.subtract)
            theta = spool.tile([P, nf], F32, tag="theta")
            V.tensor_tensor(theta[:], tmp[:], rinw[:], ALU.mult)
            omt = spool.tile([P, nf], F32, tag="omt")
            A.activation(omt[:], theta[:], AF.Copy, scale=-1.0, bias=1.0)
            t1mt = spool.tile([P, nf], F32, tag="t1mt")
            V.tensor_tensor(t1mt[:], theta[:], omt[:], ALU.mult)
            th2 = spool.tile([P, nf], F32, tag="th2")
            V.tensor_tensor(th2[:], theta[:], theta[:], ALU.mult)
            omt2 = spool.tile([P, nf], F32, tag="omt2")
            G.tensor_tensor(omt2[:], omt[:], omt[:], ALU.mult)
            p1_ = spool.tile([P, nf], F32, tag="p1_")
            V.tensor_tensor(p1_[:], indel[:], th2[:], ALU.mult)
            p2_ = spool.tile([P, nf], F32, tag="p2_")
            V.tensor_tensor(p2_[:], d01[:, :, 0], t1mt[:], ALU.mult)
            ni = spool.tile([P, nf], F32, tag="ni")
            G.tensor_tensor(ni[:], p1_[:], p2_[:], ALU.add)
            num = spool.tile([P, nf], F32, tag="num")
            V.tensor_tensor(num[:], inwh[:, :, 1], ni[:], ALU.mult)
            sden = spool.tile([P, nf], F32, tag="sden")
            G.tensor_tensor(sden[:], d01[:, :, 0], d01[:, :, 1], ALU.add)
            s2d = spool.tile([P, nf], F32, tag="s2d")
            V.scalar_tensor_tensor(s2d[:], indel[:], -2.0, sden[:], ALU.mult,
                                   ALU.add)
            c3 = spool.tile([P, nf], F32, tag="c3")
            V.tensor_tensor(c3[:], s2d[:], t1mt[:], ALU.mult)
            den = spool.tile([P, nf], F32, tag="den")
            G.tensor_tensor(den[:], indel[:], c3[:], ALU.add)
            rden = spool.tile([P, nf], F32, tag="rden")
            V.reciprocal(rden[:], den[:])
            q = spool.tile([P, nf], F32, tag="q")
            V.tensor_tensor(q[:], num[:], rden[:], ALU.mult)
            y = spool.tile([P, nf], F32, tag="y")
            G.tensor_tensor(y[:], inch[:], q[:], ALU.add)
            # log abs det
            e1 = spool.tile([P, nf], F32, tag="e1")
            V.tensor_tensor(e1[:], d01[:, :, 1], th2[:], ALU.mult)
            e2 = spool.tile([P, nf], F32, tag="e2")
            V.scalar_tensor_tensor(e2[:], indel[:], 2.0, t1mt[:], ALU.mult,
                                   ALU.mult)
            e3 = spool.tile([P, nf], F32, tag="e3")
            G.tensor_tensor(e3[:], d01[:, :, 0], omt2[:], ALU.mult)
            s3 = spool.tile([P, nf], F32, tag="s3")
            G.tensor_tensor(s3[:], e1[:], e2[:], ALU.add)
            s4 = spool.tile([P, nf], F32, tag="s4")
            V.tensor_tensor(s4[:], s3[:], e3[:], ALU.add)
            dsq = spool.tile([P, nf], F32, tag="dsq")
            V.tensor_tensor(dsq[:], indel[:], indel[:], ALU.mult)
            dnum = spool.tile([P, nf], F32, tag="dnum")
            V.tensor_tensor(dnum[:], dsq[:], s4[:], ALU.mult)
            rden2 = spool.tile([P, nf], F32, tag="rden2")
            G.tensor_tensor(rden2[:], rden[:], rden[:], ALU.mult)
            larg = spool.tile([P, nf], F32, tag="larg")
            V.tensor_tensor(larg[:], dnum[:], rden2[:], ALU.mult)
            lad = spool.tile([P, nf], F32, tag="lad")
            A.activation(lad[:], larg[:], AF.Ln)
            # inside mask + select
            absx = spool.tile([P, nf], F32, tag="absx")
            V.tensor_scalar(absx[:].bitcast(I32), x_ap.bitcast(I32),
                            0x7FFFFFFF, None, ALU.bitwise_and)
            ins = spool.tile([P, nf], F32, tag="ins")
            V.tensor_scalar(ins[:], absx[:], TB, None, ALU.is_le)
            ymx = spool.tile([P, nf], F32, tag="ymx")
            G.tensor_tensor(ymx[:], y[:], x_ap, ALU.subtract)
            ymm = spool.tile([P, nf], F32, tag="ymm")
            V.tensor_tensor(ymm[:], ymx[:], ins[:], ALU.mult)
            V.tensor_tensor(y_out_ap, ymm[:], x_ap, ALU.add)
            V.tensor_tensor(ld_tile[:], lad[:], ins[:], ALU.mult)

        # ------------- megatile loop -------------
        for t in range(NMT):
            row0 = t * NT

            def dram_a(x, ncols):
                return x[row0:row0 + NT, :].rearrange("(j p) k -> p j k", p=P)

            wi_a = apool.tile([P, nf, 2], F32, tag="wi_a")
            nc.sync.dma_start(wi_a[:], dram_a(wi, 2))
            xm = apool.tile([P, nf, 32], F32, tag="xm")
            xr = apool.tile([P, nf, 7], F32, tag="xr")
            cond_a = dram_a(cond, 10)
            nc.sync.dma_start(xm[:, :, 9:11], cond_a[:, :, 8:10])
            nc.sync.dma_start(xm[:, :, 31:32], cond_a[:, :, 0:1])
            nc.sync.dma_start(xr[:, :, 0:7], cond_a[:, :, 1:8])
            nc.sync.dma_start(xm[:, :, 0:1],
                              dram_a(wi, 2)[:, :, 1:2])

            wiout = apool.tile([P, nf, 2], F32, tag="wiout")
            ld0 = spool.tile([P, nf], F32, tag="ld0")
            ld1 = spool.tile([P, nf], F32, tag="ld1")

            # pass 1 features
            cond_chains(xm[:])
            sincos_chains(xm[:, :, 0], xm[:], 1)
            xmB = bpool.tile([P, M], F32, tag="xmB")
            xrB = bpool.tile([28, M], F32, tag="xrB")
            transpose_to_b(xm[:], xr[:], xmB, xrB)
            h1B = bpool.tile([P, M], F32, tag="h1B")
            mlp(xmB, xrB, ct["w0m_1"], ct["w0r_1"], ct["w1d_1"],
                ct["b0c_1"], ct["b1c_1"], h1B)
            # p0 MLP on cond_e rows (reuses pass-1 packed xin; rows 0..8 of
            # each block are zeroed in the weights)
            hpB = bpool.tile([P, M], F32, tag="hpB")
            for s in range(NS):
                sl = slice(512 * s, 512 * (s + 1))
                hpp = psm.tile([P, 512], F32, tag="mm")
                nc.tensor.matmul(hpp[:], ct["p0m"][:], xmB[:, sl], start=True,
                                 stop=False)
                nc.tensor.matmul(hpp[:], ct["p0r"][:], xrB[:, sl], start=False,
                                 stop=True)
                A.activation(hpB[:, sl], hpp[:], AF.Prelu,
                             bias=ct["b0cp"][:, 0:1], alpha=0.01)
            retA = apool.tile([P, nf, 4], F32, tag="retA")
            for g in range(NG):
                rpp = ptp.tile([P, 16], F32, tag="ret", bufs=1)
                nc.tensor.matmul(rpp[:], hpB[:, 128 * g:128 * (g + 1)],
                                 ct["p0w1"][:], start=True, stop=True)
                V.tensor_tensor(retA[:, 4 * g:4 * (g + 1), :],
                                rpp[:].rearrange("p (a b) -> p a b", b=4),
                                bc(ct["b1pc"][:].unsqueeze(1), (P, 4, 4)),
                                ALU.add)

            rqs_pass("p1", h1B, ct["w2b_1"], ct["eb2_1"], ct["b2u_1"],
                     wi_a[:, :, 0], wiout[:, :, 0], ld0, xm)

            # pass 2: overwrite xin col 0 with y0, recompute pe features
            V.tensor_copy(xm[:, :, 0:1], wiout[:, :, 0:1])
            sincos_chains(xm[:, :, 0], xm[:], 1)
            xmB2 = bpool.tile([P, M], F32, tag="xmB", name="xmB2")
            transpose_to_b(xm[:], xr[:], xmB2, xrB)
            h1B2 = bpool.tile([P, M], F32, tag="h1B", name="h1B2")
            mlp(xmB2, xrB, ct["w0m_2"], ct["w0r_2"], ct["w1d_2"],
                ct["b0c_2"], ct["b1c_2"], h1B2)
            rqs_pass("p2", h1B2, ct["w2b_2"], ct["eb2_2"], ct["b2u_2"],
                     wi_a[:, :, 1], wiout[:, :, 1], ld1, xm)

            # base density + final log pdf
            els = spool.tile([P, nf, 2], F32, tag="els")
            A.activation(els[:], retA[:, :, 2:4], AF.Exp, scale=-1.0)
            z = spool.tile([P, nf, 2], F32, tag="z")
            G.tensor_tensor(z[:], wiout[:], retA[:, :, 0:2], ALU.subtract)
            zz = spool.tile([P, nf, 2], F32, tag="zz")
            V.tensor_tensor(zz[:], z[:], els[:], ALU.mult)
            zz2 = spool.tile([P, nf, 2], F32, tag="zz2")
            V.tensor_tensor(zz2[:], zz[:], zz[:], ALU.mult)
            rr = spool.tile([P, nf], F32, tag="rr")
            G.tensor_tensor(rr[:], zz2[:, :, 0], zz2[:, :, 1], ALU.add)
            s1f = spool.tile([P, nf], F32, tag="s1f")
            G.tensor_tensor(s1f[:], retA[:, :, 2], retA[:, :, 3], ALU.add)
            s2f = spool.tile([P, nf], F32, tag="s2f")
            V.scalar_tensor_tensor(s2f[:], rr[:], 0.5, s1f[:], ALU.mult,
                                   ALU.add)
            l01 = spool.tile([P, nf], F32, tag="l01")
            G.tensor_tensor(l01[:], ld0[:], ld1[:], ALU.add)
            s4f = spool.tile([P, nf], F32, tag="s4f")
            V.tensor_tensor(s4f[:], l01[:], s2f[:], ALU.subtract)
            lp = spool.tile([P, nf], F32, tag="lp")
            A.activation(lp[:], s4f[:], AF.Copy, bias=-LOG2PI)

            nc.sync.dma_start(
                wi_out[row0:row0 + NT, :].rearrange("(j p) k -> p j k", p=P),
                wiout[:])
            nc.sync.dma_start(
                log_pdf[row0:row0 + NT, :].rearrange("(j p) k -> p j k", p=P),
                lp[:].unsqueeze(2))

    nc.compile()
    return nc


_NC_CACHE = {}


def get_nc(npc, nf):
    key = (npc, nf)
    if key not in _NC_CACHE:
        _NC_CACHE[key] = build_nc(npc, nf)
    return _NC_CACHE[key]


def kernel(wi, cond, p0_w0, p0_b0, p0_w1, p0_b1,
           m1_w0, m1_b0, m1_w1, m1_b1, m1_w2, m1_b2,
           m0_w0, m0_b0, m0_w1, m0_b1, m0_w2, m0_b2,
           nf=64, ncores=NCORES):
    inp = dict(p0_w0=p0_w0, p0_b0=p0_b0, p0_w1=p0_w1, p0_b1=p0_b1,
               m1_w0=m1_w0, m1_b0=m1_b0, m1_w1=m1_w1, m1_b1=m1_b1,
               m1_w2=m1_w2, m1_b2=m1_b2,
               m0_w0=m0_w0, m0_b0=m0_b0, m0_w1=m0_w1, m0_b1=m0_b1,
               m0_w2=m0_w2, m0_b2=m0_b2)
    wi = f32(wi)
    cond = f32(cond)
    n = wi.shape[0]
    npc = n // ncores
    consts = host_consts(inp)
    consts["rstc"] = reset_mask(nf)

    nc = get_nc(npc, nf)
    in_maps = []
    for c in range(ncores):
        m = {"wi": wi[c * npc:(c + 1) * npc],
             "cond": cond[c * npc:(c + 1) * npc]}
        m.update(consts)
        in_maps.append(m)
    res = run_bass_kernel_spmd(nc, in_maps, list(range(ncores))).results
    wi_out = np.concatenate([res[c]["wi_out"] for c in range(ncores)], 0)
    log_pdf = np.concatenate([res[c]["log_pdf"] for c in range(ncores)], 0)
    return wi_out, log_pdf
